# revision 5
# baseline (speedup 1.0000x reference)
"""Trainium2 Bass kernel for nn_ODEFunc_interaction (gnn_message_passing) — v2.

Math (see reference):
  dz_dt = tanh([z, t] @ vW1 + vb1) @ vW2 + vb2                    (v-net, all rows)
  for each pair (perm[2i], perm[2i+1]):
      d_i  = z[perm[2i]] - z[perm[2i+1]]
      q_i  = pW1 @ (pW2[:,0] * tanh(d_i@pW1 + pb1)^2)
      out[perm[2i]]   = dz_dt[perm[2i]]   + q_i - c0
      out[perm[2i+1]] = dz_dt[perm[2i+1]] - q_i + c0     (c0 = pW1 @ pW2[:,0])
  last 3 rows (triple) handled on host in float64 (tiny).

v2 strategy (vs the 86 us baseline which ran matmuls sequentially per
32-row chunk and tanh in 8 small ACT ops per 1024 columns):
  - Split-pair packing: each core owns 25000 gathered rows = 12500 pairs,
    4 chunks x 3125 pairs. X[128, 6272] fp16 where partition 32j+d holds
    dim d of chunk j; columns [0:3136) are pair a-members, [3136:6272) are
    the b-members (11 zero pad cols per half). Pair diff and +/-q are then
    contiguous step-1 column ops.
  - tile_position packed matmuls: h and pa as 4 concurrent row-tiles
    (K=32, M=128), dz and q as 4 concurrent col-tiles (K=128, M=32) that
    write one [128, N] psum tile in 32-partition strips. +q / -q bursts
    ACCUMULATE into the dz psum tiles, so no DVE combine pass is needed.
  - ACT (the wall at ~31 us stream time): one tanh per burst at FD=4*256
    to amortize the ~200-350 cyc per-op overhead.
  - fp16 output (halves output DMA); DVE copies psum->sbuf; GPSIMD does
    the pair diff.
"""

import os
import numpy as np

B, D, H = 200003, 32, 128
NCORES = 8
P2 = 200000            # rows covered by pairs
RPC = P2 // NCORES     # 25000 rows per core
PAIRS = RPC // 2       # 12500 pairs per core
NCHUNK = 4
PPC = PAIRS // NCHUNK  # 3125 pairs per chunk
HW_ = 3136             # padded half width (= 6*512 + 64), 4B aligned
XW = 2 * HW_           # 6272 packed columns per core

NB = 512               # pair-columns per block

_CACHE = {}
LAST_RESULTS = None    # BassKernelResults of the most recent run (for test.py)


def _blocks():
    out = []
    c0 = 0
    while c0 < HW_:
        out.append((c0, min(NB, HW_ - c0)))
        c0 += NB
    return out


def build_program():
    """Build the single-core Bass/Tile program (same program runs SPMD on 8 cores)."""
    from contextlib import ExitStack
    import concourse.bacc as bacc
    import concourse.mybir as mybir
    import concourse.tile as tile

    dt = mybir.dt
    F32 = dt.float32
    F16 = dt.float16
    AF = mybir.ActivationFunctionType
    OP = mybir.AluOpType

    # Weight tensor [128, 352] fp16:
    #   w1rep[0:128]  = tile(vW1[:32], (4,1))   (row-tile j reads rows 32j:32j+32)
    #   pw1rep[128:256] = tile(pW1, (4,1))
    #   vw2[256:288]  = vW2                      (col-tile lhsT, K=128 M=32)
    #   qwp[288:320]  = (pW1 * pW2^T).T          (q = qwp^T tanh^2)
    #   qwn[320:352]  = -qwp
    nc = bacc.Bacc()
    X = nc.dram_tensor("x", [128, 2, HW_], F16, kind="ExternalInput")
    WC = nc.dram_tensor("wcat", [128, 352], F16, kind="ExternalInput")
    BC = nc.dram_tensor("bias", [128, 2], F32, kind="ExternalInput")
    O = nc.dram_tensor("out", [128, 2, HW_], F16, kind="ExternalOutput")

    with tile.TileContext(nc) as tc, ExitStack() as ctx:
        # Two pools only (pool teardown emits cross-engine barriers); buffer
        # rotation is per-tag via tile(..., tag=, bufs=).
        sb = ctx.enter_context(tc.tile_pool(name="sb", bufs=1))
        psp = ctx.enter_context(tc.tile_pool(name="psp", bufs=1, space="PSUM"))
        wpool = dfpool = upool = vpool = sqpool = sb
        hps = paps = dzps = psp

        # Dummy tanh on a zeroed scratch tile: forces walrus to place the
        # ACT_TABLE_LOAD (~1.3us) at program start, overlapping the preamble.
        scr = wpool.tile([128, 2], F16)
        nc.gpsimd.memset(scr[:], 0.0)
        nc.scalar.activation(scr[:, 1:2], scr[:, 0:1], AF.Tanh)

        xt = wpool.tile([128, 2, HW_], F16)
        # first (runt) block's slices first so the first h burst starts ASAP;
        # weights ride the Scalar hwdge queue so they don't serialize behind
        # the x-input stream on the Sync queue.
        nc.sync.dma_start(xt[:, :, 0:NB], X[:, :, 0:NB])
        wt = wpool.tile([128, 352], F16)
        nc.scalar.dma_start(wt[:], WC[:])
        bt = wpool.tile([128, 2], F32)
        nc.scalar.dma_start(bt[:], BC[:])

        bh = bt[:, 0:1]
        pb1 = bt[:, 1:2]
        vw2 = wt[:, 256:288]
        qwp = wt[:, 288:320]
        qwn = wt[:, 320:352]

        ot = wpool.tile([128, 2, HW_], F16)

        def phase1(c0, nb):
            """DMA in, pair-diff, h+pa matmul panels and all tanh/square work.

            ACT program order is hA, v1, hB, v2 so every psum refill (pp or
            pa2, both single-buffered) is covered by a tanh on the other one.
            Returns handles needed by the deferred phase2."""
            if c0 != 0:
                nc.sync.dma_start(xt[:, :, c0 : c0 + nb], X[:, :, c0 : c0 + nb])
            dft = dfpool.tile([128, NB], F16, tag="dft", bufs=2)
            nc.gpsimd.tensor_tensor(
                dft[:, :nb], xt[:, 0, c0 : c0 + nb], xt[:, 1, c0 : c0 + nb], OP.subtract
            )
            uts = []
            sqs = []
            for half in (0, 1):
                pp = hps.tile([128, NCHUNK, NB], F32, name=f"pp{half}", tag="pp")
                for j in range(NCHUNK):
                    p0 = 32 * j
                    nc.tensor.matmul(
                        pp[:, j, :nb],
                        wt[p0 : p0 + 32, 0:128],
                        xt[p0 : p0 + 32, half, c0 : c0 + nb],
                        start=True,
                        stop=True,
                        tile_position=(p0, 0),
                    )
                ut = upool.tile([128, NCHUNK, NB], F16, name=f"u{half}", tag="ut", bufs=4)
                nc.scalar.activation(ut[:, :, :nb], pp[:, :, :nb], AF.Tanh, bias=bh[:])
                uts.append(ut)
                # pa round on 2 chunks (2 banks) + its tanh + square
                rnd = half
                pa2 = paps.tile([128, 2, NB], F32, name=f"pa{rnd}", tag="pa2")
                for jj in range(2):
                    j = 2 * rnd + jj
                    p0 = 32 * j
                    nc.tensor.matmul(
                        pa2[:, jj, :nb],
                        wt[p0 : p0 + 32, 128:256],
                        dft[p0 : p0 + 32, :nb],
                        start=True,
                        stop=True,
                        tile_position=(p0, 0),
                    )
                vt = vpool.tile([128, 2, NB], F16, name=f"v{rnd}", tag="vt", bufs=2)
                nc.scalar.activation(vt[:, :, :nb], pa2[:, :, :nb], AF.Tanh, bias=pb1[:])
                sq = sqpool.tile([128, 2, NB], F16, name=f"sq{rnd}", tag="sq", bufs=4)
                nc.vector.tensor_mul(sq[:, :, :nb], vt[:, :, :nb], vt[:, :, :nb])
                sqs.append(sq)
            return (c0, nb, uts, sqs)

        def phase2(state):
            """dz + q col-bursts into the dz psum, evacuate, DMA out.

            Each half owns a full psum bank: start=True clears per bank, so
            independent accumulation groups must never share one."""
            c0, nb, uts, sqs = state
            dzt = dzps.tile([128, 2, NB], F32, name="dzt", tag="dzt")  # dzA/[:,1]=dzB
            for half in range(2):
                for j in range(NCHUNK):
                    p0 = 32 * j
                    nc.tensor.matmul(
                        dzt[p0 : p0 + 32, half, :nb],
                        vw2,
                        uts[half][:, j, :nb],
                        start=True,
                        stop=False,
                        tile_position=(0, p0),
                        skip_group_check=True,
                    )
            for half, qw in ((0, qwp), (1, qwn)):
                for rnd in range(2):
                    for jj in range(2):
                        j = 2 * rnd + jj
                        p0 = 32 * j
                        nc.tensor.matmul(
                            dzt[p0 : p0 + 32, half, :nb],
                            qw,
                            sqs[rnd][:, jj, :nb],
                            start=False,
                            stop=True,
                            tile_position=(0, p0),
                            skip_group_check=True,
                        )
            nc.vector.tensor_copy(ot[:, :, c0 : c0 + nb], dzt[:, :, :nb])
            nc.sync.dma_start(O[:, :, c0 : c0 + nb], ot[:, :, c0 : c0 + nb])

        blocks = _blocks()
        prev = None
        for i, (c0, nb) in enumerate(blocks):
            if i == len(blocks) - 1 and prev is not None:
                # final (runt) block: flush the fat phase2 first so it overlaps
                # the remaining ACT ops instead of trailing the last tanh
                phase2(prev)
                prev = None
            state = phase1(c0, nb)
            if prev is not None:
                phase2(prev)
            prev = state
        phase2(prev)

    nc.compile()
    return nc


def _prep_weights(t, vW1, vb1, vW2, vb2, pW1, pb1, pW2):
    f32 = np.float32
    t = np.asarray(t, dtype=f32).reshape(-1)[0]
    vW1 = np.asarray(vW1, dtype=f32)
    w1rep = np.tile(np.ascontiguousarray(vW1[:32]), (4, 1))            # [128,128]
    biash = (np.asarray(vb1, f32) + t * vW1[32]).reshape(128, 1).astype(f32)
    vw2 = np.ascontiguousarray(np.asarray(vW2, f32))                   # [128,32]
    pW1 = np.asarray(pW1, f32)
    pw1rep = np.tile(pW1, (4, 1))                                      # [128,128]
    pb1c = np.asarray(pb1, f32).reshape(128, 1).copy()
    w2col = np.asarray(pW2, f32).reshape(128)
    qwp = np.ascontiguousarray((pW1 * w2col[None, :]).T)               # [128,32]
    wcat = np.hstack([w1rep, pw1rep, vw2, qwp, -qwp]).astype(np.float16)
    bias = np.hstack([biash, pb1c]).astype(f32)
    # constant part of g: c0[d] = sum_k pW1[d,k]*w2[k], in the fp16 weight
    # precision actually used on device
    c0base = qwp.astype(np.float16).astype(f32).sum(axis=0)            # [32]
    return {"wcat": np.ascontiguousarray(wcat), "bias": np.ascontiguousarray(bias),
            "_c0base": c0base}


def _pack_core(zc):
    """[25000, 32] f32 -> [128, 6272] fp16: partition 32j+d = dim d of chunk j;
    col c (< 3136) = a-member of pair 3125j+c, col 3136+c = b-member."""
    out = np.zeros((128, XW), dtype=np.float16)
    a = zc[0::2].reshape(NCHUNK, PPC, 32).transpose(0, 2, 1).reshape(128, PPC)
    b = zc[1::2].reshape(NCHUNK, PPC, 32).transpose(0, 2, 1).reshape(128, PPC)
    out[:, :PPC] = a
    out[:, HW_ : HW_ + PPC] = b
    return out.reshape(128, 2, HW_)


def _unpack_core(oc):
    """[128, 2, 3136] fp16 packed -> [25000, 32] f32 (interleaving a/b members)."""
    oc = np.asarray(oc).reshape(128, XW)
    a = oc[:, :PPC].reshape(NCHUNK, 32, PPC).transpose(0, 2, 1).reshape(PAIRS, 32)
    b = oc[:, HW_ : HW_ + PPC].reshape(NCHUNK, 32, PPC).transpose(0, 2, 1).reshape(PAIRS, 32)
    out = np.empty((RPC, 32), dtype=np.float32)
    out[0::2] = a
    out[1::2] = b
    return out


def _host_triple(t, z3, vW1, vb1, vW2, vb2, pW1, pb1, pW2):
    """Exact float64 computation of the 3 leftover rows: dz_dt + triple forces."""
    f8 = np.float64
    z3 = z3.astype(f8)
    vW1 = np.asarray(vW1, f8)
    t = float(np.asarray(t).reshape(-1)[0])
    h3 = np.tanh(z3 @ vW1[:32] + t * vW1[32] + np.asarray(vb1, f8))
    dz3 = h3 @ np.asarray(vW2, f8) + np.asarray(vb2, f8)

    pW1 = np.asarray(pW1, f8)
    w2 = np.asarray(pW2, f8).reshape(128)
    d9 = (z3[:, None, :] - z3[None, :, :]).reshape(9, 32)
    u9 = np.tanh(d9 @ pW1 + np.asarray(pb1, f8))
    s9 = (1.0 - u9 * u9) * w2[None, :]
    g9 = s9 @ pW1.T                       # grad_phi rows
    f9 = (-g9).reshape(3, 3, 32)
    f9 = f9 * (1.0 - np.eye(3)[:, :, None])
    force3 = f9.sum(axis=1) * 2.0
    return (dz3 + force3).astype(np.float32)


def kernel(t, z, perm, vW1, vb1, vW2, vb2, pW1, pb1, pW2, pb2):
    from concourse.bass_utils import run_bass_kernel_spmd

    global LAST_RESULTS
    if "nc" not in _CACHE:
        _CACHE["nc"] = build_program()
    nc = _CACHE["nc"]

    z = np.asarray(z, np.float32)
    perm = np.asarray(perm)
    weights = _prep_weights(t, vW1, vb1, vW2, vb2, pW1, pb1, pW2)

    c0base = weights.pop("_c0base")
    zg = z[perm[:P2]]                       # [200000, 32] gathered pair rows
    in_maps = []
    for c in range(NCORES):
        im = {"x": _pack_core(zg[c * RPC : (c + 1) * RPC])}
        im.update(weights)
        in_maps.append(im)

    trace = bool(int(os.environ.get("KERNEL_TRACE", "0")))
    res = run_bass_kernel_spmd(nc, in_maps, list(range(NCORES)), trace=trace)
    LAST_RESULTS = res

    out = np.empty((B, 32), dtype=np.float32)
    og = np.concatenate([_unpack_core(res.results[c]["out"]) for c in range(NCORES)], axis=0)
    vb2f = np.asarray(vb2, np.float32)
    og[0::2] += (vb2f - c0base)[None, :]
    og[1::2] += (vb2f + c0base)[None, :]
    out[perm[:P2]] = og
    out[perm[P2:]] = _host_triple(t, z[perm[P2:]], vW1, vb1, vW2, vb2, pW1, pb1, pW2)
    return out
